# revision 8
# baseline (speedup 1.0000x reference)
"""Trainium2 Bass kernel for nn_Concurrent_13623636263650 (gnn_message_passing).

Math (per batch sample, N=2000 nodes, C=64):
  u      = res / ||res||_row                  (N, C)  unit rows
  raw    = u @ u.T with zeroed diag = u@u.T - I   (symmetric)
  gmax   = max(raw flat incl diag zeros), gmin = min(...)
  rng    = gmax - gmin
  rowsum = (u@t - 1 - N*gmin)/rng,  t = sum_n u_n
  d      = rowsum^-1/2
  h      = d * origin;  q = u.T@h;  sv = sum_n h_n
  x_g1   = d * ((u@q - h) - gmin*sv) / rng
  out    = tanh(M @ Wflat + origin @ bpool).T,  M[n, d*16+i] = origin[n,d]*x_g1[n,i]

v2: whole PE pipeline in fp16 (fp32 runs LOW_HIGH 2-pass, ~3x slower);
min/max scan via tensor_scalar+accum_out (4x DVE mode) into per-chunk slots;
t via ones-matmul; input DMA split across the two HWDGE queues.

Sharding: batch 16 across 8 cores (2 samples per core), SPMD program.
"""

import numpy as np
from contextlib import ExitStack

import concourse.bass as bass
import concourse.bacc as bacc
import concourse.tile as tile
from concourse import mybir
from concourse import bass_isa
from concourse.masks import make_identity, make_upper_triangular

B, NN, C = 16, 2000, 64
F, O = 16, 32
NCORES = 8
SPC = B // NCORES          # samples per core

FP32 = mybir.dt.float32
FP16 = mybir.dt.float16
AX = mybir.AxisListType
AL = mybir.AluOpType
AF = mybir.ActivationFunctionType


def ap_view(sl, dims):
    """AP over slice `sl` keeping its partition dim/offset, with explicit
    [stride, count] free dims (element units; stride 0 broadcasts)."""
    return bass.AP(tensor=sl.tensor, offset=sl.offset, ap=[sl.ap[0]] + list(dims))


def build_program(nc, n_nodes=NN, spc=SPC):
    P = 125
    NCH = n_nodes // P           # node chunks
    FREE = 500 if n_nodes % 500 == 0 else n_nodes
    NT = n_nodes // FREE         # gram free-dim tiles
    RATIO = FREE // P            # P-chunks per free tile
    NG = NCH // RATIO            # chunk groups (of RATIO chunks)
    assert P * NCH == n_nodes and FREE * NT == n_nodes and RATIO * NT == NCH

    res_d = nc.dram_tensor("res", [spc, n_nodes, C], FP32, kind="ExternalInput").ap()
    org_d = nc.dram_tensor("origin", [spc, n_nodes, F], FP32, kind="ExternalInput").ap()
    wp_d = nc.dram_tensor("wpool", [F, F, O], FP32, kind="ExternalInput").ap()
    bp_d = nc.dram_tensor("bpool", [F, O], FP32, kind="ExternalInput").ap()
    out_d = nc.dram_tensor("out", [spc, O, n_nodes], FP32, kind="ExternalOutput").ap()

    with tile.TileContext(nc) as tc, ExitStack() as ctx:
        consts = ctx.enter_context(tc.tile_pool(name="consts", bufs=1))
        big = ctx.enter_context(tc.tile_pool(name="big", bufs=1))
        scal = ctx.enter_context(tc.tile_pool(name="scal", bufs=1))

        # ---------------- Phase A: loads & constants ----------------
        u_nc = big.tile([P, spc, NCH, C], FP32)     # res (fp32 staging)
        res_r = res_d.rearrange("s (i p) c -> p s i c", p=P)
        # split the big load across both HWDGE queues (sync + scalar)
        nc.sync.dma_start(out=u_nc[:, 0], in_=res_r[:, 0])
        nc.scalar.dma_start(out=u_nc[:, 1], in_=res_r[:, 1])
        or2 = big.tile([P, spc, NCH, F], FP32)
        nc.sync.dma_start(out=or2, in_=org_d.rearrange("s (i p) c -> p s i c", p=P))
        w2f = consts.tile([128, 2, O], FP32)         # f-tile k rows: (d%8)*16+i
        nc.sync.dma_start(out=w2f, in_=wp_d.rearrange("(k d) i o -> (d i) k o", k=2))
        bpf = consts.tile([F, O], FP32)
        nc.sync.dma_start(out=bpf, in_=bp_d)
        w2 = consts.tile([128, 2, O], FP16)
        nc.vector.tensor_copy(w2, w2f)
        bp = consts.tile([F, O], FP16)
        nc.vector.tensor_copy(bp, bpf)

        ident = consts.tile([P, P], FP16)
        make_identity(nc, ident)
        wedge = consts.tile([P, P], FP16)
        make_upper_triangular(nc, wedge, val=1.0, diag=False)
        ones_p = consts.tile([P, 1], FP16)
        nc.vector.memset(ones_p, 1.0)

        # ---------------- Phase B: row norms + normalized fp16 u ----------------
        sq = big.tile([P, spc * NCH * C], FP32)      # scratch
        u_fl = u_nc.rearrange("p s i c -> p (s i c)")
        nc.vector.tensor_tensor(out=sq, in0=u_fl, in1=u_fl, op=AL.mult)
        nrm2 = scal.tile([P, spc, NCH], FP32)
        nc.vector.tensor_reduce(nrm2, sq.rearrange("p (s i c) -> p s i c", s=spc, i=NCH),
                                axis=AX.X, op=AL.add)
        rn = scal.tile([P, spc, NCH], FP32)
        nc.scalar.activation(out=rn, in_=nrm2, func=AF.Sqrt)
        nc.vector.reciprocal(out=rn, in_=rn)
        u16 = big.tile([P, spc, NCH, C], FP16)
        nc.vector.tensor_tensor(
            out=u16, in0=u_nc,
            in1=ap_view(rn, [[NCH, spc], [1, NCH], [0, C]]), op=AL.mult)
        o16 = big.tile([P, spc, NCH, F], FP16)
        nc.vector.tensor_copy(o16, or2)

        # ---------------- Phase C: uT & oT via PE transposes (fp16) ----------------
        uT = big.tile([64, spc, n_nodes], FP16)      # per-sample, base partition 0
        oT = big.tile([F, spc, n_nodes], FP16)
        with tc.tile_pool(name="pc", bufs=2, space="PSUM") as pc:
            for g in range(NG):
                # fp16 PSUM writes must be 4B-aligned: pad each transpose
                # to a 128-wide slot, then strided-copy the 125 real cols.
                tps = pc.tile([64, spc, RATIO, 128], FP16, tag="tps")
                ops_ = pc.tile([F, spc, RATIO, 128], FP16, tag="ops")
                for k in range(RATIO):
                    i = g * RATIO + k
                    for s in range(spc):
                        nc.tensor.transpose(
                            tps[:, s, k, 0:P],
                            u16[:, s, i, :], ident)
                        nc.tensor.transpose(
                            ops_[:, s, k, 0:P],
                            o16[:, s, i, :], ident)
                nc.scalar.copy(
                    uT[:, :, g * FREE:(g + 1) * FREE].rearrange(
                        "c s (r p) -> c s r p", p=P),
                    tps[:, :, :, 0:P])
                nc.vector.tensor_copy(
                    oT[:, :, g * FREE:(g + 1) * FREE].rearrange(
                        "c s (r p) -> c s r p", p=P),
                    ops_[:, :, :, 0:P])

        # ---------------- Phase E: gram (fp16) + tensor_scalar 4x scans ----------------
        gmax_r = scal.tile([P, spc], FP32)           # replicated per-sample scalars
        gmin_r = scal.tile([P, spc], FP32)           # holds -gmin after negate
        mx_sl = scal.tile([P, spc * NCH], FP32)      # per-chunk max slots
        mn_sl = scal.tile([P, spc * NCH], FP32)
        with tc.tile_pool(name="pg", bufs=2, space="PSUM") as pg, \
             tc.tile_pool(name="sg", bufs=3) as sg:
            for idx in range(spc * NCH):
                s, i = idx % spc, idx // spc
                js = i // RATIO
                d_off = P * (i % RATIO)
                so = d_off - (d_off % 2)             # even start for 4x mode
                wi = (FREE - so) + FREE * (NT - 1 - js)   # even width
                gps = pg.tile([P, NT, 512], FP32, tag="gps")
                for j in range(js, NT):
                    nc.tensor.matmul(
                        gps[:, j, 0:FREE],
                        uT[:, s, i * P:(i + 1) * P],
                        uT[:, s, j * FREE:(j + 1) * FREE],
                        start=True, stop=True)
                g16 = sg.tile([P, n_nodes], FP16, tag="g16")
                nc.scalar.activation(out=g16[:, 0:FREE - so],
                                     in_=gps[:, js, so:FREE], func=AF.Copy)
                nfull = NT - 1 - js
                if nfull > 0:
                    nc.scalar.activation(
                        out=g16[:, FREE - so:wi].rearrange(
                            "p (t w) -> p t w", w=FREE),
                        in_=gps[:, js + 1:NT, 0:FREE], func=AF.Copy)
                # mask diag block (cols so+?..): diag block at offset d_off-so
                db = d_off - so
                nc.vector.tensor_tensor(out=g16[:, db:db + P], in0=g16[:, db:db + P],
                                        in1=wedge, op=AL.mult)
                scx = sg.tile([P, n_nodes], FP16, tag="scx")
                scn = sg.tile([P, n_nodes], FP16, tag="scn")
                nc.vector.tensor_scalar(
                    out=scx[:, 0:wi], in0=g16[:, 0:wi], scalar1=-2.0, scalar2=None,
                    op0=AL.max, op1=AL.max, accum_out=mx_sl[:, idx:idx + 1])
                nc.vector.tensor_scalar(
                    out=scn[:, 0:wi], in0=g16[:, 0:wi], scalar1=2.0, scalar2=None,
                    op0=AL.min, op1=AL.min, accum_out=mn_sl[:, idx:idx + 1])
            for s in range(spc):
                mx_p = scal.tile([P, 1], FP32, tag="mxp")
                mn_p = scal.tile([P, 1], FP32, tag="mnp")
                sl_v = mx_sl.rearrange("p (i s) -> p s i", s=spc)
                nc.vector.tensor_reduce(mx_p, sl_v[:, s], axis=AX.X, op=AL.max)
                sl_n = mn_sl.rearrange("p (i s) -> p s i", s=spc)
                nc.vector.tensor_reduce(mn_p, sl_n[:, s], axis=AX.X, op=AL.min)
                nc.vector.tensor_scalar(out=mx_p, in0=mx_p, scalar1=0.0, scalar2=None,
                                        op0=AL.max)
                nc.vector.tensor_scalar(out=mn_p, in0=mn_p, scalar1=0.0, scalar2=-1.0,
                                        op0=AL.min, op1=AL.mult)
                nc.gpsimd.partition_all_reduce(gmax_r[:, s:s + 1], mx_p, channels=P,
                                               reduce_op=bass_isa.ReduceOp.max)
                nc.gpsimd.partition_all_reduce(gmin_r[:, s:s + 1], mn_p, channels=P,
                                               reduce_op=bass_isa.ReduceOp.max)

        inv_r = scal.tile([P, spc], FP32)            # 1/rng; note gmin_r = -gmin
        nc.vector.tensor_tensor(out=inv_r, in0=gmax_r, in1=gmin_r, op=AL.add)
        nc.vector.reciprocal(out=inv_r, in_=inv_r)
        neg_gmin = gmin_r                            # alias for clarity

        # ---------------- Phase F: t (ones-matmul), rowsum, d ----------------
        t2 = scal.tile([64, spc], FP16)
        d2 = scal.tile([P, spc, NCH], FP32)
        with tc.tile_pool(name="pf", bufs=1, space="PSUM") as pf, \
             tc.tile_pool(name="sf", bufs=1) as sf:
            t_ps = pf.tile([1, spc, C], FP32)
            for s in range(spc):
                for i in range(NCH):
                    nc.tensor.matmul(t_ps[:, s, :], ones_p, u16[:, s, i, :],
                                     start=(i == 0), stop=(i == NCH - 1))
            t_sb = sf.tile([1, spc, C], FP16)
            nc.vector.tensor_copy(t_sb, t_ps)
            tT_ps = pf.tile([64, spc, 2], FP16)
            for s in range(spc):
                nc.tensor.transpose(tT_ps[:, s, 0:1], t_sb[:, s, :],
                                    ident[0:1, 0:1])
            nc.vector.tensor_copy(t2, tT_ps[:, :, 0])
            rs_ps = pf.tile([P, spc, NCH], FP32)
            for s in range(spc):
                for i in range(NCH):
                    nc.tensor.matmul(rs_ps[:, s, i:i + 1],
                                     uT[:, s, i * P:(i + 1) * P],
                                     t2[:, s:s + 1], start=True, stop=True)
            # rowsum_norm = (rs - 1 + N*neg_gmin) * inv;  d = 1/sqrt(rowsum_norm)
            bv = scal.tile([P, spc], FP32)
            nc.vector.tensor_scalar(out=bv, in0=neg_gmin, scalar1=float(n_nodes),
                                    scalar2=-1.0, op0=AL.mult, op1=AL.add)
            nc.vector.tensor_tensor(out=bv, in0=bv, in1=inv_r, op=AL.mult)
            for s in range(spc):
                nc.scalar.activation(out=d2[:, s, :], in_=rs_ps[:, s, :], func=AF.Sqrt,
                                     scale=inv_r[:, s:s + 1], bias=bv[:, s:s + 1])
        nc.vector.reciprocal(out=d2, in_=d2)

        # ---------------- Phase G: h, q, sv ----------------
        h2 = big.tile([P, spc, NCH, F], FP16)
        nc.vector.tensor_tensor(out=h2, in0=o16,
                                in1=ap_view(d2, [[NCH, spc], [1, NCH], [0, F]]),
                                op=AL.mult)
        q_sb = scal.tile([64, spc, F], FP16)
        with tc.tile_pool(name="pq", bufs=1, space="PSUM") as pq:
            q_ps = pq.tile([64, spc, F], FP32)
            for s in range(spc):
                for i in range(NCH):
                    nc.tensor.matmul(q_ps[:, s, :], u16[:, s, i, :],
                                     h2[:, s, i, :], start=(i == 0), stop=(i == NCH - 1))
            nc.vector.tensor_copy(q_sb, q_ps)
        sv_t = scal.tile([P, spc, F], FP32)
        nc.vector.tensor_reduce(
            sv_t, ap_view(h2, [[NCH * F, spc], [1, F], [F, NCH]]), axis=AX.X, op=AL.add)
        sv_r = scal.tile([P, spc, F], FP32)
        nc.gpsimd.partition_all_reduce(
            sv_r.rearrange("p s f -> p (s f)"), sv_t.rearrange("p s f -> p (s f)"),
            channels=P, reduce_op=bass_isa.ReduceOp.add)

        # ---------------- Phase H: v, x_g1 ----------------
        xg1 = big.tile([P, spc, NCH, F], FP16)
        with tc.tile_pool(name="pv", bufs=2, space="PSUM") as pv:
            for s in range(spc):
                v_ps = pv.tile([P, NCH, F], FP32, tag="vps")
                for i in range(NCH):
                    nc.tensor.matmul(v_ps[:, i, :],
                                     uT[:, s, i * P:(i + 1) * P],
                                     q_sb[:, s, :], start=True, stop=True)
                gsv = scal.tile([P, F], FP32, tag="gsv")   # gmin*sv = -(neg_gmin*sv)
                nc.vector.tensor_scalar(out=gsv, in0=sv_r[:, s, :],
                                        scalar1=neg_gmin[:, s:s + 1], scalar2=None,
                                        op0=AL.mult)
                # xg1 = ((v - h) + neg_gmin*sv) * (d*inv)
                nc.vector.tensor_tensor(out=xg1[:, s], in0=v_ps, in1=h2[:, s],
                                        op=AL.subtract)
                nc.vector.tensor_tensor(out=xg1[:, s], in0=xg1[:, s],
                                        in1=ap_view(gsv, [[0, NCH], [1, F]]),
                                        op=AL.add)
                dsc = scal.tile([P, NCH], FP32, tag="dsc")
                nc.vector.tensor_scalar(out=dsc, in0=d2[:, s, :],
                                        scalar1=inv_r[:, s:s + 1], scalar2=None,
                                        op0=AL.mult)
                nc.vector.tensor_tensor(out=xg1[:, s], in0=xg1[:, s],
                                        in1=ap_view(dsc, [[1, NCH], [0, F]]),
                                        op=AL.mult)

        # ---------------- Phase I+J: M build, transpose, final matmuls ----------------
        with tc.tile_pool(name="pm", bufs=2, space="PSUM") as pm, \
             tc.tile_pool(name="po", bufs=2, space="PSUM") as po, \
             tc.tile_pool(name="sm", bufs=2) as sm:
            for s in range(spc):
                mt0 = sm.tile([128, n_nodes], FP16, tag="mt0")
                mt1 = sm.tile([128, n_nodes], FP16, tag="mt1")
                for g in range(NG):
                    mg = sm.tile([P, RATIO, F, F], FP16, tag="mg")
                    # M[p, k, d, i] = origin[p, s, g*R+k, d] * xg1[p, s, g*R+k, i]
                    nc.gpsimd.tensor_tensor(
                        out=mg,
                        in0=ap_view(o16[:, s, g * RATIO:(g + 1) * RATIO, :],
                                    [[F, RATIO], [1, F], [0, F]]),
                        in1=ap_view(xg1[:, s, g * RATIO:(g + 1) * RATIO, :],
                                    [[F, RATIO], [0, F], [1, F]]),
                        op=AL.mult)
                    mtp0 = pm.tile([128, RATIO, 128], FP16, tag="mtp0")
                    mtp1 = pm.tile([128, RATIO, 128], FP16, tag="mtp1")
                    for k in range(RATIO):
                        mg_f = mg[:, k].rearrange("p d i -> p (d i)")
                        nc.tensor.transpose(mtp0[:, k, 0:P],
                                            mg_f[:, 0:128], ident)
                        nc.tensor.transpose(mtp1[:, k, 0:P],
                                            mg_f[:, 128:256], ident)
                    nc.scalar.copy(
                        mt0[:, g * FREE:(g + 1) * FREE].rearrange(
                            "q (r p) -> q r p", p=P),
                        mtp0[:, :, 0:P])
                    nc.vector.tensor_copy(
                        mt1[:, g * FREE:(g + 1) * FREE].rearrange(
                            "q (r p) -> q r p", p=P),
                        mtp1[:, :, 0:P])
                for k in range(NT):
                    ob = po.tile([O, FREE], FP32, tag="ob")
                    sl = slice(k * FREE, (k + 1) * FREE)
                    nc.tensor.matmul(ob, w2[:, 0, :],
                                     mt0[:, sl], start=True, stop=False)
                    nc.tensor.matmul(ob, w2[:, 1, :],
                                     mt1[:, sl], start=False, stop=False)
                    nc.tensor.matmul(ob, bp,
                                     oT[:, s, sl], start=False, stop=True)
                    osb = sm.tile([O, FREE], FP32, tag="osb")
                    nc.scalar.activation(out=osb, in_=ob, func=AF.Tanh)
                    nc.sync.dma_start(out=out_d[s, :, sl], in_=osb)
    return nc


_PROGRAM = None


def _get_program():
    global _PROGRAM
    if _PROGRAM is None:
        nc = bacc.Bacc("TRN2", target_bir_lowering=False, debug=False,
                       num_devices=NCORES)
        build_program(nc)
        nc.compile()
        _PROGRAM = nc
    return _PROGRAM


def kernel(**inputs):
    from concourse.bass_utils import run_bass_kernel_spmd
    res = np.asarray(inputs["res_x"], dtype=np.float32)
    org = np.asarray(inputs["origin_x"], dtype=np.float32)
    wp = np.asarray(inputs["weights_pool_x"], dtype=np.float32)
    bpl = np.asarray(inputs["bias_pool_x"], dtype=np.float32)
    nc = _get_program()
    in_maps = [
        {"res": res[c * SPC:(c + 1) * SPC], "origin": org[c * SPC:(c + 1) * SPC],
         "wpool": wp, "bpool": bpl}
        for c in range(NCORES)
    ]
    r = run_bass_kernel_spmd(nc, in_maps, list(range(NCORES)))
    out = np.concatenate([r.results[c]["out"] for c in range(NCORES)], axis=0)
    return out.astype(np.float32)


# revision 14
# speedup vs baseline: 1.1717x; 1.1717x over previous
"""Trainium2 Bass kernel for nn_Concurrent_13623636263650 (gnn_message_passing).

Math (per batch sample, N=2000 nodes, C=64):
  u      = res / ||res||_row                  (N, C)  unit rows
  raw    = u @ u.T with zeroed diag = u@u.T - I   (symmetric)
  gmax   = max(raw flat incl diag zeros), gmin = min(...)
  rng    = gmax - gmin
  rowsum = (u@t - 1 - N*gmin)/rng,  t = sum_n u_n
  d      = rowsum^-1/2
  h      = d * origin;  q = u.T@h;  sv = sum_n h_n
  x_g1   = d * ((u@q - h) - gmin*sv) / rng
  out    = tanh(M @ Wflat + origin @ bpool).T,  M[n, d*16+i] = origin[n,d]*x_g1[n,i]

v2: whole PE pipeline in fp16 (fp32 runs LOW_HIGH 2-pass, ~3x slower);
min/max scan via tensor_scalar+accum_out (4x DVE mode) into per-chunk slots;
t via ones-matmul; input DMA split across the two HWDGE queues.

Sharding: batch 16 across 8 cores (2 samples per core), SPMD program.
"""

import numpy as np
from contextlib import ExitStack

import concourse.bass as bass
import concourse.bacc as bacc
import concourse.tile as tile
from concourse import mybir
from concourse import bass_isa
from concourse.masks import make_identity, make_upper_triangular

B, NN, C = 16, 2000, 64
F, O = 16, 32
NCORES = 8
SPC = B // NCORES          # samples per core

FP32 = mybir.dt.float32
FP16 = mybir.dt.float16
AX = mybir.AxisListType
AL = mybir.AluOpType
AF = mybir.ActivationFunctionType


def ap_view(sl, dims):
    """AP over slice `sl` keeping its partition dim/offset, with explicit
    [stride, count] free dims (element units; stride 0 broadcasts)."""
    return bass.AP(tensor=sl.tensor, offset=sl.offset, ap=[sl.ap[0]] + list(dims))


def build_program(nc, n_nodes=NN, spc=SPC):
    P = 125
    NCH = n_nodes // P           # node chunks
    FREE = 500 if n_nodes % 500 == 0 else n_nodes
    NT = n_nodes // FREE         # gram free-dim tiles
    RATIO = FREE // P            # P-chunks per free tile
    NG = NCH // RATIO            # chunk groups (of RATIO chunks)
    assert P * NCH == n_nodes and FREE * NT == n_nodes and RATIO * NT == NCH

    res_d = nc.dram_tensor("res", [spc, n_nodes, C], FP32, kind="ExternalInput").ap()
    org_d = nc.dram_tensor("origin", [spc, n_nodes, F], FP32, kind="ExternalInput").ap()
    wp_d = nc.dram_tensor("wpool", [F, F, O], FP32, kind="ExternalInput").ap()
    bp_d = nc.dram_tensor("bpool", [F, O], FP32, kind="ExternalInput").ap()
    out_d = nc.dram_tensor("out", [spc, O, n_nodes], FP32, kind="ExternalOutput").ap()

    with tile.TileContext(nc) as tc, ExitStack() as ctx:
        consts = ctx.enter_context(tc.tile_pool(name="consts", bufs=1))
        big = ctx.enter_context(tc.tile_pool(name="big", bufs=1))
        scal = ctx.enter_context(tc.tile_pool(name="scal", bufs=1))

        # ---------------- Phase A: loads & constants ----------------
        u_nc = big.tile([P, spc, NCH, C], FP32)     # res (fp32 staging)
        res_r = res_d.rearrange("s (i p) c -> p s i c", p=P)
        # split the big load across both HWDGE queues (sync + scalar)
        nc.sync.dma_start(out=u_nc[:, 0], in_=res_r[:, 0])
        nc.scalar.dma_start(out=u_nc[:, 1], in_=res_r[:, 1])
        or2 = big.tile([P, spc, NCH, F], FP32)
        nc.sync.dma_start(out=or2, in_=org_d.rearrange("s (i p) c -> p s i c", p=P))
        w2f = consts.tile([128, 2, O], FP32)         # f-tile k rows: (d%8)*16+i
        nc.sync.dma_start(out=w2f, in_=wp_d.rearrange("(k d) i o -> (d i) k o", k=2))
        bpf = consts.tile([F, O], FP32)
        nc.sync.dma_start(out=bpf, in_=bp_d)
        w2 = consts.tile([128, 2, O], FP16)
        nc.vector.tensor_copy(w2, w2f)
        bp = consts.tile([F, O], FP16)
        nc.vector.tensor_copy(bp, bpf)

        ident = consts.tile([P, P], FP16)
        make_identity(nc, ident)
        # shifted -I blocks (one per within-tile chunk position): accumulated
        # onto the gram's diagonal tile to cancel the self-similarity diag
        neg_sh = consts.tile([P, RATIO, FREE], FP16)
        nc.gpsimd.memset(neg_sh, 0.0)
        for r in range(RATIO):
            nc.gpsimd.affine_select(
                out=neg_sh[:, r, :], in_=neg_sh[:, r, :],
                compare_op=AL.not_equal, fill=-1.0,
                base=r * P, pattern=[[-1, FREE]], channel_multiplier=1)
        ones_p = consts.tile([P, 1], FP16)
        nc.vector.memset(ones_p, 1.0)

        # ---------------- Phase B: row norms + normalized fp16 u ----------------
        sq = big.tile([P, spc * NCH * C], FP32)      # scratch
        u_fl = u_nc.rearrange("p s i c -> p (s i c)")
        nc.vector.tensor_tensor(out=sq, in0=u_fl, in1=u_fl, op=AL.mult)
        nrm2 = scal.tile([P, spc, NCH], FP32)
        nc.vector.tensor_reduce(nrm2, sq.rearrange("p (s i c) -> p s i c", s=spc, i=NCH),
                                axis=AX.X, op=AL.add)
        rn = scal.tile([P, spc, NCH], FP32)
        nc.scalar.activation(out=rn, in_=nrm2, func=AF.Sqrt)
        nc.vector.reciprocal(out=rn, in_=rn)
        u16 = big.tile([P, spc, NCH, C], FP16)
        nc.vector.tensor_tensor(
            out=u16, in0=u_nc,
            in1=ap_view(rn, [[NCH, spc], [1, NCH], [0, C]]), op=AL.mult)
        o16 = big.tile([P, spc, NCH, F], FP16)
        nc.vector.tensor_copy(o16, or2)

        # ---------------- Phase C: uT & oT via PE transposes (fp16) ----------------
        uT = big.tile([64, spc, n_nodes], FP16)      # per-sample, base partition 0
        oT = big.tile([F, spc, n_nodes], FP16)
        with tc.tile_pool(name="pc", bufs=2, space="PSUM") as pc:
            for g in range(NG):
                # fp16 PSUM writes must be 4B-aligned: pad each transpose
                # to a 128-wide slot, then strided-copy the 125 real cols.
                tps = pc.tile([64, spc, RATIO, 128], FP16, tag="tps")
                ops_ = pc.tile([F, spc, RATIO, 128], FP16, tag="ops")
                for k in range(RATIO):
                    i = g * RATIO + k
                    for s in range(spc):
                        nc.tensor.transpose(
                            tps[:, s, k, 0:P],
                            u16[:, s, i, :], ident)
                        nc.tensor.transpose(
                            ops_[:, s, k, 0:P],
                            o16[:, s, i, :], ident)
                nc.scalar.copy(
                    uT[:, :, g * FREE:(g + 1) * FREE].rearrange(
                        "c s (r p) -> c s r p", p=P),
                    tps[:, :, :, 0:P])
                nc.vector.tensor_copy(
                    oT[:, :, g * FREE:(g + 1) * FREE].rearrange(
                        "c s (r p) -> c s r p", p=P),
                    ops_[:, :, :, 0:P])

        # ---------------- Phase E: gram (fp16) + tensor_scalar 4x scans ----------------
        gmax_r = scal.tile([P, spc], FP32)           # replicated per-sample scalars
        gmin_r = scal.tile([P, spc], FP32)           # holds -gmin after negate
        with tc.tile_pool(name="pg", bufs=2, space="PSUM") as pg, \
             tc.tile_pool(name="sg", bufs=3) as sg, \
             tc.tile_pool(name="sacc", bufs=2) as sacc:
            # Two samples' accumulator chains interleave on DVE so one
            # sample's scan fills the other's cast/matmul latency bubbles.
            accs = []
            for s in range(spc):
                acc_mx = sacc.tile([P, n_nodes], FP16, tag="amx")
                acc_mn = sacc.tile([P, n_nodes], FP16, tag="amn")
                nc.vector.memset(acc_mx, -2.0)
                nc.vector.memset(acc_mn, 2.0)
                accs.append((acc_mx, acc_mn))
            for idx in range(spc * NCH):
                s, i = idx % spc, idx // spc
                acc_mx, acc_mn = accs[s]
                js = i // RATIO
                d_off = P * (i % RATIO)
                wi = (FREE - d_off) + FREE * (NT - 1 - js)
                gps = pg.tile([P, NT, 512], FP32, tag="gps")
                nc.tensor.matmul(
                    gps[:, js, 0:FREE],
                    uT[:, s, i * P:(i + 1) * P],
                    uT[:, s, js * FREE:(js + 1) * FREE],
                    start=True, stop=False)
                # cancel the diagonal: accumulate a shifted -I onto the
                # diag tile (same [0:FREE] region completes the group)
                nc.tensor.matmul(
                    gps[:, js, 0:FREE], ident, neg_sh[:, i % RATIO, :],
                    start=False, stop=True)
                for j in range(js + 1, NT):
                    nc.tensor.matmul(
                        gps[:, j, 0:FREE],
                        uT[:, s, i * P:(i + 1) * P],
                        uT[:, s, j * FREE:(j + 1) * FREE],
                        start=True, stop=True)
                g16 = sg.tile([P, n_nodes], FP16, tag="g16")
                nc.scalar.activation(out=g16[:, 0:FREE - d_off],
                                     in_=gps[:, js, d_off:FREE], func=AF.Copy)
                nfull = NT - 1 - js
                if nfull > 0:
                    nc.scalar.activation(
                        out=g16[:, FREE - d_off:wi].rearrange(
                            "p (t w) -> p t w", w=FREE),
                        in_=gps[:, js + 1:NT, 0:FREE], func=AF.Copy)
                nc.vector.tensor_tensor(out=acc_mx[:, 0:wi], in0=acc_mx[:, 0:wi],
                                        in1=g16[:, 0:wi], op=AL.max)
                nc.vector.tensor_tensor(out=acc_mn[:, 0:wi], in0=acc_mn[:, 0:wi],
                                        in1=g16[:, 0:wi], op=AL.min)
            for s in range(spc):
                acc_mx, acc_mn = accs[s]
                mx_p = scal.tile([P, 1], FP32, tag="mxp")
                mn_p = scal.tile([P, 1], FP32, tag="mnp")
                nc.vector.tensor_reduce(mx_p, acc_mx, axis=AX.X, op=AL.max)
                nc.vector.tensor_reduce(mn_p, acc_mn, axis=AX.X, op=AL.min)
                nc.vector.tensor_scalar(out=mx_p, in0=mx_p, scalar1=0.0, scalar2=None,
                                        op0=AL.max)
                nc.vector.tensor_scalar(out=mn_p, in0=mn_p, scalar1=0.0, scalar2=-1.0,
                                        op0=AL.min, op1=AL.mult)
                nc.gpsimd.partition_all_reduce(gmax_r[:, s:s + 1], mx_p, channels=P,
                                               reduce_op=bass_isa.ReduceOp.max)
                nc.gpsimd.partition_all_reduce(gmin_r[:, s:s + 1], mn_p, channels=P,
                                               reduce_op=bass_isa.ReduceOp.max)

        inv_r = scal.tile([P, spc], FP32)            # 1/rng; note gmin_r = -gmin
        nc.vector.tensor_tensor(out=inv_r, in0=gmax_r, in1=gmin_r, op=AL.add)
        nc.vector.reciprocal(out=inv_r, in_=inv_r)
        neg_gmin = gmin_r                            # alias for clarity

        # ---------------- Phase F: t (ones-matmul), rowsum, d ----------------
        t2 = scal.tile([64, spc], FP16)
        d2 = scal.tile([P, spc, NCH], FP32)
        with tc.tile_pool(name="pf", bufs=1, space="PSUM") as pf, \
             tc.tile_pool(name="sf", bufs=1) as sf:
            t_ps = pf.tile([1, spc, C], FP32)
            for s in range(spc):
                for i in range(NCH):
                    nc.tensor.matmul(t_ps[:, s, :], ones_p, u16[:, s, i, :],
                                     start=(i == 0), stop=(i == NCH - 1))
            t_sb = sf.tile([1, spc, C], FP16)
            nc.vector.tensor_copy(t_sb, t_ps)
            tT_ps = pf.tile([64, spc, 2], FP16)
            for s in range(spc):
                nc.tensor.transpose(tT_ps[:, s, 0:1], t_sb[:, s, :],
                                    ident[0:1, 0:1])
            nc.vector.tensor_copy(t2, tT_ps[:, :, 0])
            rs_ps = pf.tile([P, spc, NCH], FP32)
            for s in range(spc):
                for i in range(NCH):
                    nc.tensor.matmul(rs_ps[:, s, i:i + 1],
                                     uT[:, s, i * P:(i + 1) * P],
                                     t2[:, s:s + 1], start=True, stop=True)
            # rowsum_norm = (rs - 1 + N*neg_gmin) * inv;  d = 1/sqrt(rowsum_norm)
            bv = scal.tile([P, spc], FP32)
            nc.vector.tensor_scalar(out=bv, in0=neg_gmin, scalar1=float(n_nodes),
                                    scalar2=-1.0, op0=AL.mult, op1=AL.add)
            nc.vector.tensor_tensor(out=bv, in0=bv, in1=inv_r, op=AL.mult)
            for s in range(spc):
                nc.scalar.activation(out=d2[:, s, :], in_=rs_ps[:, s, :], func=AF.Sqrt,
                                     scale=inv_r[:, s:s + 1], bias=bv[:, s:s + 1])
        nc.vector.reciprocal(out=d2, in_=d2)

        # ---------------- Phase G: h, q, sv ----------------
        h2 = big.tile([P, spc, NCH, F], FP16)
        nc.vector.tensor_tensor(out=h2, in0=o16,
                                in1=ap_view(d2, [[NCH, spc], [1, NCH], [0, F]]),
                                op=AL.mult)
        q_sb = scal.tile([64, spc, F], FP16)
        with tc.tile_pool(name="pq", bufs=1, space="PSUM") as pq:
            q_ps = pq.tile([64, spc, F], FP32)
            for s in range(spc):
                for i in range(NCH):
                    nc.tensor.matmul(q_ps[:, s, :], u16[:, s, i, :],
                                     h2[:, s, i, :], start=(i == 0), stop=(i == NCH - 1))
            nc.vector.tensor_copy(q_sb, q_ps)
        sv_t = scal.tile([P, spc, F], FP32)
        nc.vector.tensor_reduce(
            sv_t, ap_view(h2, [[NCH * F, spc], [1, F], [F, NCH]]), axis=AX.X, op=AL.add)
        sv_r = scal.tile([P, spc, F], FP32)
        nc.gpsimd.partition_all_reduce(
            sv_r.rearrange("p s f -> p (s f)"), sv_t.rearrange("p s f -> p (s f)"),
            channels=P, reduce_op=bass_isa.ReduceOp.add)

        # ---------------- Phase H: v, x_g1 ----------------
        xg1 = big.tile([P, spc, NCH, F], FP16)
        with tc.tile_pool(name="pv", bufs=2, space="PSUM") as pv:
            for s in range(spc):
                v_ps = pv.tile([P, NCH, F], FP32, tag="vps")
                for i in range(NCH):
                    nc.tensor.matmul(v_ps[:, i, :],
                                     uT[:, s, i * P:(i + 1) * P],
                                     q_sb[:, s, :], start=True, stop=True)
                gsv = scal.tile([P, F], FP32, tag="gsv")   # gmin*sv = -(neg_gmin*sv)
                nc.vector.tensor_scalar(out=gsv, in0=sv_r[:, s, :],
                                        scalar1=neg_gmin[:, s:s + 1], scalar2=None,
                                        op0=AL.mult)
                # xg1 = ((v - h) + neg_gmin*sv) * (d*inv)
                nc.vector.tensor_tensor(out=xg1[:, s], in0=v_ps, in1=h2[:, s],
                                        op=AL.subtract)
                nc.vector.tensor_tensor(out=xg1[:, s], in0=xg1[:, s],
                                        in1=ap_view(gsv, [[0, NCH], [1, F]]),
                                        op=AL.add)
                dsc = scal.tile([P, NCH], FP32, tag="dsc")
                nc.vector.tensor_scalar(out=dsc, in0=d2[:, s, :],
                                        scalar1=inv_r[:, s:s + 1], scalar2=None,
                                        op0=AL.mult)
                nc.vector.tensor_tensor(out=xg1[:, s], in0=xg1[:, s],
                                        in1=ap_view(dsc, [[1, NCH], [0, F]]),
                                        op=AL.mult)

        # ---------------- Phase I+J: M build, transpose, final matmuls ----------------
        with tc.tile_pool(name="pm", bufs=2, space="PSUM") as pm, \
             tc.tile_pool(name="po", bufs=2, space="PSUM") as po, \
             tc.tile_pool(name="sm", bufs=2) as sm:
            for s in range(spc):
                mt0 = sm.tile([128, n_nodes], FP16, tag="mt0")
                mt1 = sm.tile([128, n_nodes], FP16, tag="mt1")
                for g in range(NG):
                    mg = sm.tile([P, RATIO, F, F], FP16, tag="mg")
                    # M[p, k, d, i] = origin[p, s, g*R+k, d] * xg1[p, s, g*R+k, i]
                    nc.gpsimd.tensor_tensor(
                        out=mg,
                        in0=ap_view(o16[:, s, g * RATIO:(g + 1) * RATIO, :],
                                    [[F, RATIO], [1, F], [0, F]]),
                        in1=ap_view(xg1[:, s, g * RATIO:(g + 1) * RATIO, :],
                                    [[F, RATIO], [0, F], [1, F]]),
                        op=AL.mult)
                    mtp0 = pm.tile([128, RATIO, 128], FP16, tag="mtp0")
                    mtp1 = pm.tile([128, RATIO, 128], FP16, tag="mtp1")
                    for k in range(RATIO):
                        mg_f = mg[:, k].rearrange("p d i -> p (d i)")
                        nc.tensor.transpose(mtp0[:, k, 0:P],
                                            mg_f[:, 0:128], ident)
                        nc.tensor.transpose(mtp1[:, k, 0:P],
                                            mg_f[:, 128:256], ident)
                    nc.scalar.copy(
                        mt0[:, g * FREE:(g + 1) * FREE].rearrange(
                            "q (r p) -> q r p", p=P),
                        mtp0[:, :, 0:P])
                    nc.vector.tensor_copy(
                        mt1[:, g * FREE:(g + 1) * FREE].rearrange(
                            "q (r p) -> q r p", p=P),
                        mtp1[:, :, 0:P])
                for k in range(NT):
                    ob = po.tile([O, FREE], FP32, tag="ob")
                    sl = slice(k * FREE, (k + 1) * FREE)
                    nc.tensor.matmul(ob, w2[:, 0, :],
                                     mt0[:, sl], start=True, stop=False)
                    nc.tensor.matmul(ob, w2[:, 1, :],
                                     mt1[:, sl], start=False, stop=False)
                    nc.tensor.matmul(ob, bp,
                                     oT[:, s, sl], start=False, stop=True)
                    osb = sm.tile([O, FREE], FP32, tag="osb")
                    nc.scalar.activation(out=osb, in_=ob, func=AF.Tanh)
                    nc.sync.dma_start(out=out_d[s, :, sl], in_=osb)
    return nc


_PROGRAM = None


def _get_program():
    global _PROGRAM
    if _PROGRAM is None:
        nc = bacc.Bacc("TRN2", target_bir_lowering=False, debug=False,
                       num_devices=NCORES)
        build_program(nc)
        nc.compile()
        _PROGRAM = nc
    return _PROGRAM


def kernel(**inputs):
    from concourse.bass_utils import run_bass_kernel_spmd
    res = np.asarray(inputs["res_x"], dtype=np.float32)
    org = np.asarray(inputs["origin_x"], dtype=np.float32)
    wp = np.asarray(inputs["weights_pool_x"], dtype=np.float32)
    bpl = np.asarray(inputs["bias_pool_x"], dtype=np.float32)
    nc = _get_program()
    in_maps = [
        {"res": res[c * SPC:(c + 1) * SPC], "origin": org[c * SPC:(c + 1) * SPC],
         "wpool": wp, "bpool": bpl}
        for c in range(NCORES)
    ]
    r = run_bass_kernel_spmd(nc, in_maps, list(range(NCORES)))
    out = np.concatenate([r.results[c]["out"] for c in range(NCORES)], axis=0)
    return out.astype(np.float32)
